# revision 4
# baseline (speedup 1.0000x reference)
"""Bilateral filter (nn_BilateralFilter) on 8 Trainium2 NeuronCores.

Sharding: data-parallel over (batch, H-half): core i -> sample i//2,
row-half i%2 (128 output rows each). Each core receives a host-padded
input slab [C, 132, 260] (2-row/2-col zero halos) plus per-sample tap
weights derived from `params` on the host; it computes the 5x5 (or
masked 3x3) bilateral filter for its 128x256 output tile.

Math (exact rewrite of the reference):
  out[c,p] = sum_t sk[t]*e_t[p]*x[c,p+t] / sum_t (sk[t]+1e-8*mask[t])*e_t[p]
  e_t[p]   = exp(-((m*s)[p+t] - (m*s)[p])^2),  s = 1/(sqrt(2)*sigma2)
where m is the channel-mean image and sk is the mask-folded normalized
spatial kernel. The 1e-8*mask term reproduces the reference's
`w/(w.sum()+1e-8)` epsilon after multiplying through by the color-kernel
normalizer.

Engine split: per-pixel weights rule out TensorE (no shared operand), so
the 25-tap MAC stream is elementwise; channels [0:C0] run on VectorE and
[C0:C] on GPSIMD concurrently. ScalarE computes the Square/Exp chain.
"""

import numpy as np

B, C, H, W = 4, 32, 256, 256
HALF = H // 2          # output rows per core
SLAB_H = HALF + 4      # input rows incl. 2-row halos
SLAB_W = W + 4         # input cols incl. 2-col halos
NCORES = 8
NT = 25                # 5x5 taps
C0 = 22                # channels on VectorE; rest on GPSIMD

_CACHE = {}


def _host_tap_constants(params):
    """Per-sample sk_eff[25], sk2[25], s2c scalar (all float32 math)."""
    p = params.astype(np.float32)
    sig = (1.0 / (1.0 + np.exp(-p))).astype(np.float32)
    coords = (np.arange(5, dtype=np.float32) - 2.0)
    grid = coords[:, None] ** 2 + coords[None, :] ** 2
    center3 = ((np.abs(coords)[:, None] <= 1) & (np.abs(coords)[None, :] <= 1)).astype(np.float32)
    out = []
    for b in range(B):
        k_raw = np.float32(1.0) + np.float32(2.0) * sig[b, 0]
        is5 = bool(k_raw >= 2.0)
        sigma1 = np.float32(3.5) + np.float32(5.5) * sig[b, 1]
        sigma2 = np.float32(5.5) + np.float32(7.5) * sig[b, 2]
        mask = np.ones((5, 5), np.float32) if is5 else center3
        sk = np.exp(-grid / (2.0 * sigma1 ** 2)).astype(np.float32) * mask
        sk = (sk / sk.sum()).astype(np.float32)
        sk_eff = sk.reshape(NT)
        sk2 = (sk_eff + np.float32(1e-8) * mask.reshape(NT)).astype(np.float32)
        # m_s = (sum_c x) * s2c  ==  mean * (1/(sqrt(2)*sigma2))
        s2c = np.float32(1.0 / (np.sqrt(2.0, dtype=np.float64) * float(sigma2)) / C)
        out.append((sk_eff, sk2, s2c, is5))
    return out


def _build(active_taps, n_iter=1, c0=C0):
    from contextlib import ExitStack, nullcontext
    import concourse.tile as tile
    from concourse import bacc, mybir

    f32 = mybir.dt.float32
    AF = mybir.ActivationFunctionType
    AL = mybir.AluOpType
    c1 = C - c0  # gpsimd channels

    nc = bacc.Bacc("TRN2", target_bir_lowering=False, debug=False,
                   num_devices=NCORES)
    xs_d = nc.dram_tensor("xs", [C, SLAB_H, SLAB_W], f32, kind="ExternalInput").ap()
    cst_d = nc.dram_tensor("cst", [128, 51], f32, kind="ExternalInput").ap()
    out_d = nc.dram_tensor("out", [C, HALF, W], f32, kind="ExternalOutput").ap()

    with tile.TileContext(nc) as tc, ExitStack() as ctx:
        loop_ctx = tc.For_i(0, n_iter, 1) if n_iter > 1 else nullcontext()
        pool_c = ctx.enter_context(tc.tile_pool(name="cstp", bufs=1))
        pool_x = ctx.enter_context(tc.tile_pool(name="xp", bufs=2))
        pool_mean = ctx.enter_context(tc.tile_pool(name="meanp", bufs=1))
        pool_w = ctx.enter_context(tc.tile_pool(name="wp", bufs=3))
        pool_acc = ctx.enter_context(tc.tile_pool(name="accp", bufs=1))
        pool_tmp = ctx.enter_context(tc.tile_pool(name="tmpp", bufs=1))

        cst = pool_c.tile([128, 51], f32, name="cst")
        nc.sync.dma_start(cst[:], cst_d)
        ctx.enter_context(loop_ctx)

        # ---- x slab for di=0 (feeds the mean too) + tail rows ----
        xg = {}
        xg0 = pool_x.tile([128, C, SLAB_W], f32, tag="xg", name="xg0")
        nc.sync.dma_start(xg0[:], xs_d[:, 0:128, :].transpose([1, 0, 2]))
        xg[0] = xg0
        xt = pool_mean.tile([4, C, SLAB_W], f32, name="xt")
        nc.sync.dma_start(xt[:], xs_d[:, 128:132, :].transpose([1, 0, 2]))

        # ---- channel mean (rows 0..128 and tail rows 128..132) ----
        m_acc = pool_mean.tile([128, SLAB_W], f32, name="m_acc")
        nc.vector.tensor_reduce(
            out=m_acc[:], in_=xg[0][:].transpose([0, 2, 1]),
            axis=mybir.AxisListType.X, op=AL.add)
        mt_acc = pool_mean.tile([4, SLAB_W], f32, name="mt_acc")
        nc.vector.tensor_reduce(
            out=mt_acc[:], in_=xt[:].transpose([0, 2, 1]),
            axis=mybir.AxisListType.X, op=AL.add)

        # scaled mean m_s = (sum_c x) * s2c   (cst col 50)
        m_sA = pool_mean.tile([128, SLAB_W], f32, name="m_sA")
        nc.vector.tensor_scalar_mul(out=m_sA[:], in0=m_acc[:], scalar1=cst[:, 50:51])
        m_sB = pool_mean.tile([4, SLAB_W], f32, name="m_sB")
        nc.vector.tensor_scalar_mul(out=m_sB[:], in0=mt_acc[:], scalar1=cst[0:4, 50:51])

        # di-shifted views of m_s (rows di..di+128 of the slab)
        msd = {0: m_sA}
        for di in range(1, 5):
            t = pool_mean.tile([128, SLAB_W], f32, tag=f"msd{di}", name=f"msd{di}")
            nc.sync.dma_start(t[0:128 - di, :], m_sA[di:128, :])
            nc.sync.dma_start(t[128 - di:128, :], m_sB[0:di, :])
            msd[di] = t

        # ---- main tap loop ----
        acc_d = pool_acc.tile([128, c0, W], f32, name="acc_d")
        acc_g = pool_acc.tile([128, c1, W], f32, name="acc_g")
        denom = pool_acc.tile([128, W], f32, name="denom")
        first = True
        for di in range(5):
            if di > 0 and any((di * 5 + dj) in active_taps for dj in range(5)):
                t = pool_x.tile([128, C, SLAB_W], f32, tag="xg", name=f"xg{di}")
                nc.sync.dma_start(t[:], xs_d[:, di:di + 128, :].transpose([1, 0, 2]))
                xg[di] = t
            for dj in range(5):
                t_idx = di * 5 + dj
                if t_idx not in active_taps:
                    continue
                d = pool_w.tile([128, W], f32, tag="d", name=f"d{t_idx}")
                nc.vector.tensor_tensor(
                    out=d[:], in0=msd[di][:, dj:dj + W], in1=msd[2][:, 2:2 + W],
                    op=AL.subtract)
                sq = pool_w.tile([128, W], f32, tag="sq", name=f"sq{t_idx}")
                nc.scalar.activation(out=sq[:], in_=d[:], func=AF.Square)
                e = pool_w.tile([128, W], f32, tag="e", name=f"e{t_idx}")
                nc.scalar.activation(out=e[:], in_=sq[:], func=AF.Exp, scale=-1.0)
                Wt = pool_w.tile([128, W], f32, tag="Wt", name=f"Wt{t_idx}")
                nc.vector.tensor_scalar_mul(
                    out=Wt[:], in0=e[:], scalar1=cst[:, t_idx:t_idx + 1])
                if first:
                    nc.vector.tensor_scalar_mul(
                        out=denom[:], in0=e[:], scalar1=cst[:, 25 + t_idx:26 + t_idx])
                else:
                    nc.vector.scalar_tensor_tensor(
                        out=denom[:], in0=e[:], scalar=cst[:, 25 + t_idx:26 + t_idx],
                        in1=denom[:], op0=AL.mult, op1=AL.add)
                Wb_d = Wt[:].unsqueeze(1).broadcast_to([128, c0, W])
                Wb_g = Wt[:].unsqueeze(1).broadcast_to([128, c1, W])
                xsl_d = xg[di][:, 0:c0, dj:dj + W]
                xsl_g = xg[di][:, c0:C, dj:dj + W]
                if first:
                    nc.vector.tensor_tensor(out=acc_d[:], in0=Wb_d, in1=xsl_d, op=AL.mult)
                    nc.gpsimd.tensor_tensor(out=acc_g[:], in0=Wb_g, in1=xsl_g, op=AL.mult)
                else:
                    prod_d = pool_tmp.tile([128, c0, W], f32, tag="prod_d", name=f"pd{t_idx}")
                    nc.vector.tensor_tensor(out=prod_d[:], in0=Wb_d, in1=xsl_d, op=AL.mult)
                    nc.vector.tensor_add(acc_d[:], acc_d[:], prod_d[:])
                    prod_g = pool_tmp.tile([128, c1, W], f32, tag="prod_g", name=f"pg{t_idx}")
                    nc.gpsimd.tensor_tensor(out=prod_g[:], in0=Wb_g, in1=xsl_g, op=AL.mult)
                    nc.gpsimd.tensor_tensor(out=acc_g[:], in0=acc_g[:], in1=prod_g[:], op=AL.add)
                first = False

        # ---- normalize + store ----
        recip = pool_w.tile([128, W], f32, tag="recip", name="recip")
        nc.vector.reciprocal(out=recip[:], in_=denom[:])
        og_d = pool_tmp.tile([128, c0, W], f32, tag="prod_d", name="og_d")
        nc.vector.tensor_tensor(
            out=og_d[:], in0=acc_d[:],
            in1=recip[:].unsqueeze(1).broadcast_to([128, c0, W]), op=AL.mult)
        nc.sync.dma_start(out_d[0:c0, :, :].transpose([1, 0, 2]), og_d[:])
        og_g = pool_tmp.tile([128, c1, W], f32, tag="prod_g", name="og_g")
        nc.gpsimd.tensor_tensor(
            out=og_g[:], in0=acc_g[:],
            in1=recip[:].unsqueeze(1).broadcast_to([128, c1, W]), op=AL.mult)
        nc.sync.dma_start(out_d[c0:C, :, :].transpose([1, 0, 2]), og_g[:])

    nc.compile()
    return nc


def _prep_inputs(x, params):
    """Build per-core in_maps."""
    x = np.ascontiguousarray(x, dtype=np.float32)
    tap_consts = _host_tap_constants(params)
    active = set()
    for (sk_eff, sk2, s2c, is5) in tap_consts:
        active |= {t for t in range(NT) if sk2[t] != 0.0}
    # pad whole batch once: [B, C, H+4, W+4]
    xp = np.pad(x, ((0, 0), (0, 0), (2, 2), (2, 2)))
    in_maps = []
    for core in range(NCORES):
        b, half = core // 2, core % 2
        h0 = half * HALF
        slab = np.ascontiguousarray(xp[b, :, h0:h0 + SLAB_H, :])
        sk_eff, sk2, s2c, _ = tap_consts[b]
        cst = np.zeros((128, 51), np.float32)
        cst[:, 0:25] = sk_eff[None, :]
        cst[:, 25:50] = sk2[None, :]
        cst[:, 50] = s2c
        in_maps.append({"xs": slab, "cst": cst})
    return in_maps, frozenset(active)


def kernel(x, params, n_iter=1, c0=C0):
    from concourse.bass_utils import run_bass_kernel_spmd
    in_maps, active = _prep_inputs(x, params)
    key = ("nc", active, n_iter, c0)
    if key not in _CACHE:
        _CACHE[key] = _build(active, n_iter, c0)
    nc = _CACHE[key]
    res = run_bass_kernel_spmd(nc, in_maps, list(range(NCORES)))
    out = np.empty((B, C, H, W), np.float32)
    for core in range(NCORES):
        b, half = core // 2, core % 2
        out[b, :, half * HALF:(half + 1) * HALF, :] = res.results[core]["out"]
    return out


# revision 10
# speedup vs baseline: 1.8272x; 1.8272x over previous
"""Bilateral filter (nn_BilateralFilter) on 8 Trainium2 NeuronCores.

Sharding: data-parallel over (batch, H-half): core i -> sample i//2,
row-half i%2 (128 output rows each). Each core receives a host-padded
input slab [C, 132, 260] (2-row/2-col zero halos) plus per-sample tap
weights derived from `params` on the host; it computes the 5x5 (or
masked 3x3) bilateral filter for its 128x256 output tile.

Math (exact rewrite of the reference):
  out[c,p] = sum_t sk[t]*e_t[p]*x[c,p+t] / sum_t (sk[t]+1e-8*mask[t])*e_t[p]
  e_t[p]   = exp(-((m*s)[p+t] - (m*s)[p])^2),  s = 1/(sqrt(2)*sigma2)
where m is the channel-mean image and sk is the mask-folded normalized
spatial kernel. The 1e-8*mask term reproduces the reference's
`w/(w.sum()+1e-8)` epsilon after multiplying through by the color-kernel
normalizer.

Engine split: per-pixel weights rule out TensorE (no shared operand), so
the 25-tap MAC stream is elementwise; channels [0:C0] run on VectorE and
[C0:C] on GPSIMD concurrently. ScalarE computes the Square/Exp chain.
"""

import numpy as np

B, C, H, W = 4, 32, 256, 256
HALF = H // 2          # output rows per core
SLAB_H = HALF + 4      # input rows incl. 2-row halos
SLAB_W = W + 4         # input cols incl. 2-col halos
NCORES = 8
NT = 25                # 5x5 taps
C0 = 22                # channels on VectorE; rest on GPSIMD

_CACHE = {}


def _host_tap_constants(params):
    """Per-sample sk_eff[25], sk2[25], s2c scalar (all float32 math)."""
    p = params.astype(np.float32)
    sig = (1.0 / (1.0 + np.exp(-p))).astype(np.float32)
    coords = (np.arange(5, dtype=np.float32) - 2.0)
    grid = coords[:, None] ** 2 + coords[None, :] ** 2
    center3 = ((np.abs(coords)[:, None] <= 1) & (np.abs(coords)[None, :] <= 1)).astype(np.float32)
    out = []
    for b in range(B):
        k_raw = np.float32(1.0) + np.float32(2.0) * sig[b, 0]
        is5 = bool(k_raw >= 2.0)
        sigma1 = np.float32(3.5) + np.float32(5.5) * sig[b, 1]
        sigma2 = np.float32(5.5) + np.float32(7.5) * sig[b, 2]
        mask = np.ones((5, 5), np.float32) if is5 else center3
        sk = np.exp(-grid / (2.0 * sigma1 ** 2)).astype(np.float32) * mask
        sk = (sk / sk.sum()).astype(np.float32)
        sk_eff = sk.reshape(NT)
        sk2 = (sk_eff + np.float32(1e-8) * mask.reshape(NT)).astype(np.float32)
        # m_s = (sum_c x) * s2c  ==  mean * (1/(sqrt(2)*sigma2))
        s2c = np.float32(1.0 / (np.sqrt(2.0, dtype=np.float64) * float(sigma2)) / C)
        out.append((sk_eff, sk2, s2c, is5))
    return out


def _build(active_taps, n_iter=1, c0=C0):
    from contextlib import ExitStack, nullcontext
    import concourse.tile as tile
    import concourse.bass as bass
    from concourse import bacc, mybir

    f32 = mybir.dt.float32
    AF = mybir.ActivationFunctionType
    AL = mybir.AluOpType
    act = sorted(active_taps)
    t_first, t_last = act[0], act[-1]

    nc = bacc.Bacc("TRN2", target_bir_lowering=False, debug=False,
                   num_devices=NCORES)
    xs_d = nc.dram_tensor("xs", [C, SLAB_H, SLAB_W], f32, kind="ExternalInput").ap()
    cst_d = nc.dram_tensor("cst", [128, 51], f32, kind="ExternalInput").ap()
    id_d = nc.dram_tensor("ident", [128, 128], f32, kind="ExternalInput").ap()
    id4_d = nc.dram_tensor("ident4", [128, 4], f32, kind="ExternalInput").ap()
    out_d = nc.dram_tensor("out", [C, HALF, W], f32, kind="ExternalOutput").ap()

    HC = C // 2  # channels per PSUM half
    NB = 8       # PSUM banks per half
    BC = HC // NB  # channels per bank (2)

    with tile.TileContext(nc) as tc, ExitStack() as ctx:
        loop_ctx = tc.For_i(0, n_iter, 1) if n_iter > 1 else nullcontext()
        pool_c = ctx.enter_context(tc.tile_pool(name="cstp", bufs=1))
        pool_x = ctx.enter_context(tc.tile_pool(name="xp", bufs=2))
        pool_mean = ctx.enter_context(tc.tile_pool(name="meanp", bufs=1))
        pool_w = ctx.enter_context(tc.tile_pool(name="wp", bufs=3))
        pool_wp = ctx.enter_context(tc.tile_pool(name="wplanes", bufs=1))
        pool_tmp = ctx.enter_context(tc.tile_pool(name="tmpp", bufs=2))
        pool_ps = ctx.enter_context(
            tc.tile_pool(name="psum", bufs=1, space=bass.MemorySpace.PSUM))

        cst = pool_c.tile([128, 51], f32, name="cst")
        nc.sync.dma_start(cst[:], cst_d)
        ident = pool_c.tile([128, 128], f32, name="ident")
        nc.sync.dma_start(ident[:], id_d)
        ident4 = pool_c.tile([128, 4], f32, name="ident4")
        nc.sync.dma_start(ident4[:], id4_d)
        ctx.enter_context(loop_ctx)

        # ---- x slab rows 0..128 (mean) + tail rows 128..132 ----
        xg0 = pool_mean.tile([128, C, SLAB_W], f32, name="xg0")
        nc.sync.dma_start(xg0[:], xs_d[:, 0:128, :].transpose([1, 0, 2]))
        xt = pool_mean.tile([128, SLAB_W], f32, name="xt")
        for r in range(4):
            nc.sync.dma_start(xt[r * 32:(r + 1) * 32, :], xs_d[:, 128 + r, :])

        # ---- channel sum on PE (identity-matmul accumulate into PSUM) ----
        ps_m = pool_ps.tile([128, SLAB_W], f32, name="ps_m")
        ps_mt = pool_ps.tile([4, SLAB_W], f32, name="ps_mt")
        for c in range(C):
            nc.tensor.matmul(ps_m[:], ident[:], xg0[:, c, :],
                             start=(c == 0), stop=(c == C - 1))
        nc.tensor.matmul(ps_mt[:], ident4[:], xt[:], start=True, stop=True)

        # scaled mean m_s = (sum_c x) * s2c   (cst col 50)
        m_sA = pool_mean.tile([128, SLAB_W], f32, name="m_sA")
        nc.vector.tensor_scalar_mul(out=m_sA[:], in0=ps_m[:], scalar1=cst[:, 50:51])
        m_sB = pool_mean.tile([4, SLAB_W], f32, name="m_sB")
        nc.vector.tensor_scalar_mul(out=m_sB[:], in0=ps_mt[:], scalar1=cst[0:4, 50:51])

        # di-shifted views of m_s (rows di..di+128 of the slab)
        msd = {0: m_sA}
        for di in range(1, 5):
            t = pool_mean.tile([128, SLAB_W], f32, tag=f"msd{di}", name=f"msd{di}")
            nc.sync.dma_start(t[0:128 - di, :], m_sA[di:128, :])
            nc.sync.dma_start(t[128 - di:128, :], m_sB[0:di, :])
            msd[di] = t

        # ---- phase A: per-tap weight planes + denominator (DVE + ACT) ----
        Wp = {}
        denom = pool_mean.tile([128, W], f32, name="denom")
        for t_idx in act:
            di, dj = divmod(t_idx, 5)
            d = pool_w.tile([128, W], f32, tag="d", name=f"d{t_idx}")
            nc.vector.tensor_tensor(
                out=d[:], in0=msd[di][:, dj:dj + W], in1=msd[2][:, 2:2 + W],
                op=AL.subtract)
            sq = pool_w.tile([128, W], f32, tag="sq", name=f"sq{t_idx}")
            nc.scalar.activation(out=sq[:], in_=d[:], func=AF.Square)
            e = pool_w.tile([128, W], f32, tag="e", name=f"e{t_idx}")
            nc.scalar.activation(out=e[:], in_=sq[:], func=AF.Exp, scale=-1.0)
            Wt = pool_wp.tile([128, W], f32, tag="Wt", bufs=len(act), name=f"Wt{t_idx}")
            nc.vector.tensor_scalar_mul(
                out=Wt[:], in0=e[:], scalar1=cst[:, t_idx:t_idx + 1])
            Wp[t_idx] = Wt
            if t_idx == t_first:
                nc.vector.tensor_scalar_mul(
                    out=denom[:], in0=e[:], scalar1=cst[:, 25 + t_idx:26 + t_idx])
            else:
                nc.vector.scalar_tensor_tensor(
                    out=denom[:], in0=e[:], scalar=cst[:, 25 + t_idx:26 + t_idx],
                    in1=denom[:], op0=AL.mult, op1=AL.add)
        recip = pool_mean.tile([128, W], f32, name="recip")
        nc.vector.reciprocal(out=recip[:], in_=denom[:])

        # ---- phase B: per-half MAC: DVE mults; adds split PE(psum)/DVE ----
        QP = 12           # channels accumulated on PE per half (6 banks)
        QD = HC - QP      # channels accumulated on DVE per half
        for half in range(2):
            ch0 = half * HC
            pb = [pool_ps.tile([128, BC, W], f32, tag=f"pb{b}", name=f"pb{half}_{b}")
                  for b in range(QP // BC)]
            acc_sb = pool_tmp.tile([128, QD, W], f32, tag="acc_sb", bufs=1, name=f"accs{half}")
            for di in range(5):
                if not any((di * 5 + dj) in active_taps for dj in range(5)):
                    continue
                xh = pool_x.tile([128, HC, SLAB_W], f32, tag="xh", name=f"xh{half}_{di}")
                nc.sync.dma_start(
                    xh[:], xs_d[ch0:ch0 + HC, di:di + 128, :].transpose([1, 0, 2]))
                for dj in range(5):
                    t_idx = di * 5 + dj
                    if t_idx not in active_taps:
                        continue
                    prod = pool_tmp.tile([128, HC, W], f32, tag="prod",
                                         name=f"pr{half}_{t_idx}")
                    nc.vector.tensor_tensor(
                        out=prod[:],
                        in0=Wp[t_idx][:].unsqueeze(1).broadcast_to([128, HC, W]),
                        in1=xh[:, :, dj:dj + W], op=AL.mult)
                    for b in range(QP // BC):
                        nc.tensor.matmul(
                            pb[b][:], ident[:], prod[:, b * BC:(b + 1) * BC, :],
                            start=(t_idx == t_first), stop=(t_idx == t_last))
                    if t_idx == t_first:
                        nc.vector.tensor_copy(acc_sb[:], prod[:, QP:HC, :])
                    else:
                        nc.vector.tensor_add(acc_sb[:], acc_sb[:], prod[:, QP:HC, :])
            og = pool_tmp.tile([128, HC, W], f32, tag="prod", name=f"og{half}")
            for b in range(QP // BC):
                nc.vector.tensor_tensor(
                    out=og[:, b * BC:(b + 1) * BC, :], in0=pb[b][:],
                    in1=recip[:].unsqueeze(1).broadcast_to([128, BC, W]),
                    op=AL.mult)
            nc.vector.tensor_tensor(
                out=og[:, QP:HC, :], in0=acc_sb[:],
                in1=recip[:].unsqueeze(1).broadcast_to([128, QD, W]), op=AL.mult)
            nc.sync.dma_start(
                out_d[ch0:ch0 + HC, :, :].transpose([1, 0, 2]), og[:])

    nc.compile()
    return nc


def _prep_inputs(x, params):
    """Build per-core in_maps."""
    x = np.ascontiguousarray(x, dtype=np.float32)
    tap_consts = _host_tap_constants(params)
    active = set()
    for (sk_eff, sk2, s2c, is5) in tap_consts:
        active |= {t for t in range(NT) if sk2[t] != 0.0}
    # pad whole batch once: [B, C, H+4, W+4]
    xp = np.pad(x, ((0, 0), (0, 0), (2, 2), (2, 2)))
    in_maps = []
    for core in range(NCORES):
        b, half = core // 2, core % 2
        h0 = half * HALF
        slab = np.ascontiguousarray(xp[b, :, h0:h0 + SLAB_H, :])
        sk_eff, sk2, s2c, _ = tap_consts[b]
        cst = np.zeros((128, 51), np.float32)
        cst[:, 0:25] = sk_eff[None, :]
        cst[:, 25:50] = sk2[None, :]
        cst[:, 50] = s2c
        in_maps.append({"xs": slab, "cst": cst,
                        "ident": np.eye(128, dtype=np.float32),
                        "ident4": np.repeat(np.eye(4, dtype=np.float32), 32, axis=0)})
    return in_maps, frozenset(active)


def kernel(x, params, n_iter=1, c0=C0):
    from concourse.bass_utils import run_bass_kernel_spmd
    in_maps, active = _prep_inputs(x, params)
    key = ("nc", active, n_iter, c0)
    if key not in _CACHE:
        _CACHE[key] = _build(active, n_iter, c0)
    nc = _CACHE[key]
    res = run_bass_kernel_spmd(nc, in_maps, list(range(NCORES)))
    out = np.empty((B, C, H, W), np.float32)
    for core in range(NCORES):
        b, half = core // 2, core % 2
        out[b, :, half * HALF:(half + 1) * HALF, :] = res.results[core]["out"]
    return out


# revision 12
# speedup vs baseline: 2.2092x; 1.2091x over previous
"""Bilateral filter (nn_BilateralFilter) on 8 Trainium2 NeuronCores.

Sharding: data-parallel over (batch, H-half): core i -> sample i//2,
row-half i%2 (128 output rows each). Each core receives a host-padded
input slab [C, 132, 260] (2-row/2-col zero halos) plus per-sample tap
weights derived from `params` on the host; it computes the 5x5 (or
masked 3x3) bilateral filter for its 128x256 output tile.

Math (exact rewrite of the reference):
  out[c,p] = sum_t sk[t]*e_t[p]*x[c,p+t] / sum_t (sk[t]+1e-8*mask[t])*e_t[p]
  e_t[p]   = exp(-((m*s)[p+t] - (m*s)[p])^2),  s = 1/(sqrt(2)*sigma2)
where m is the channel-mean image and sk is the mask-folded normalized
spatial kernel. The 1e-8*mask term reproduces the reference's
`w/(w.sum()+1e-8)` epsilon after multiplying through by the color-kernel
normalizer.

Engine split: per-pixel weights rule out TensorE (no shared operand), so
the 25-tap MAC stream is elementwise; channels [0:C0] run on VectorE and
[C0:C] on GPSIMD concurrently. ScalarE computes the Square/Exp chain.
"""

import numpy as np

B, C, H, W = 4, 32, 256, 256
HALF = H // 2          # output rows per core
SLAB_H = HALF + 4      # input rows incl. 2-row halos
SLAB_W = W + 4         # input cols incl. 2-col halos
NCORES = 8
NT = 25                # 5x5 taps
C0 = 22                # channels on VectorE; rest on GPSIMD

_CACHE = {}


def _host_tap_constants(params):
    """Per-sample sk_eff[25], sk2[25], s2c scalar (all float32 math)."""
    p = params.astype(np.float32)
    sig = (1.0 / (1.0 + np.exp(-p))).astype(np.float32)
    coords = (np.arange(5, dtype=np.float32) - 2.0)
    grid = coords[:, None] ** 2 + coords[None, :] ** 2
    center3 = ((np.abs(coords)[:, None] <= 1) & (np.abs(coords)[None, :] <= 1)).astype(np.float32)
    out = []
    for b in range(B):
        k_raw = np.float32(1.0) + np.float32(2.0) * sig[b, 0]
        is5 = bool(k_raw >= 2.0)
        sigma1 = np.float32(3.5) + np.float32(5.5) * sig[b, 1]
        sigma2 = np.float32(5.5) + np.float32(7.5) * sig[b, 2]
        mask = np.ones((5, 5), np.float32) if is5 else center3
        sk = np.exp(-grid / (2.0 * sigma1 ** 2)).astype(np.float32) * mask
        sk = (sk / sk.sum()).astype(np.float32)
        sk_eff = sk.reshape(NT)
        sk2 = (sk_eff + np.float32(1e-8) * mask.reshape(NT)).astype(np.float32)
        # m_s = (sum_c x) * s2c  ==  mean * (1/(sqrt(2)*sigma2))
        s2c = np.float32(1.0 / (np.sqrt(2.0, dtype=np.float64) * float(sigma2)) / C)
        out.append((sk_eff, sk2, s2c, is5))
    return out


def _build(active_taps, n_iter=1, c0=C0):
    from contextlib import ExitStack, nullcontext
    import concourse.tile as tile
    import concourse.bass as bass
    from concourse import bacc, mybir

    f32 = mybir.dt.float32
    AF = mybir.ActivationFunctionType
    AL = mybir.AluOpType
    act = sorted(active_taps)
    t_first, t_last = act[0], act[-1]

    nc = bacc.Bacc("TRN2", target_bir_lowering=False, debug=False,
                   num_devices=NCORES)
    xs_d = nc.dram_tensor("xs", [C, SLAB_H, SLAB_W], f32, kind="ExternalInput").ap()
    cst_d = nc.dram_tensor("cst", [128, 51], f32, kind="ExternalInput").ap()
    id_d = nc.dram_tensor("ident", [128, 128], f32, kind="ExternalInput").ap()
    id4_d = nc.dram_tensor("ident4", [128, 4], f32, kind="ExternalInput").ap()
    out_d = nc.dram_tensor("out", [C, HALF, W], f32, kind="ExternalOutput").ap()

    HC = C // 2  # channels per PSUM half
    NB = 8       # PSUM banks per half
    BC = HC // NB  # channels per bank (2)

    with tile.TileContext(nc) as tc, ExitStack() as ctx:
        loop_ctx = tc.For_i(0, n_iter, 1) if n_iter > 1 else nullcontext()
        pool_c = ctx.enter_context(tc.tile_pool(name="cstp", bufs=1))
        pool_x = ctx.enter_context(tc.tile_pool(name="xp", bufs=2))
        pool_mean = ctx.enter_context(tc.tile_pool(name="meanp", bufs=1))
        pool_w = ctx.enter_context(tc.tile_pool(name="wp", bufs=3))
        pool_wp = ctx.enter_context(tc.tile_pool(name="wplanes", bufs=1))
        pool_tmp = ctx.enter_context(tc.tile_pool(name="tmpp", bufs=2))
        pool_ps = ctx.enter_context(
            tc.tile_pool(name="psum", bufs=1, space=bass.MemorySpace.PSUM))

        cst = pool_c.tile([128, 51], f32, name="cst")
        nc.sync.dma_start(cst[:], cst_d)
        ident = pool_c.tile([128, 128], f32, name="ident")
        nc.sync.dma_start(ident[:], id_d)
        ident4 = pool_c.tile([128, 4], f32, name="ident4")
        nc.sync.dma_start(ident4[:], id4_d)
        ctx.enter_context(loop_ctx)

        # ---- x slab rows 0..128 (mean) + tail rows 128..132 ----
        xg0 = pool_mean.tile([128, C, SLAB_W], f32, name="xg0")
        nc.sync.dma_start(xg0[:], xs_d[:, 0:128, :].transpose([1, 0, 2]))
        xt = pool_mean.tile([128, SLAB_W], f32, name="xt")
        for r in range(4):
            nc.sync.dma_start(xt[r * 32:(r + 1) * 32, :], xs_d[:, 128 + r, :])

        # ---- channel sum on PE (identity-matmul accumulate into PSUM) ----
        ps_m = pool_ps.tile([128, SLAB_W], f32, name="ps_m")
        ps_mt = pool_ps.tile([4, SLAB_W], f32, name="ps_mt")
        for c in range(C):
            nc.tensor.matmul(ps_m[:], ident[:], xg0[:, c, :],
                             start=(c == 0), stop=(c == C - 1))
        nc.tensor.matmul(ps_mt[:], ident4[:], xt[:], start=True, stop=True)

        # scaled mean m_s = (sum_c x) * s2c   (cst col 50)
        m_sA = pool_mean.tile([128, SLAB_W], f32, name="m_sA")
        nc.vector.tensor_scalar_mul(out=m_sA[:], in0=ps_m[:], scalar1=cst[:, 50:51])
        m_sB = pool_mean.tile([4, SLAB_W], f32, name="m_sB")
        nc.vector.tensor_scalar_mul(out=m_sB[:], in0=ps_mt[:], scalar1=cst[0:4, 50:51])

        # di-shifted views of m_s (rows di..di+128 of the slab)
        msd = {0: m_sA}
        for di in range(1, 5):
            t = pool_mean.tile([128, SLAB_W], f32, tag=f"msd{di}", name=f"msd{di}")
            nc.sync.dma_start(t[0:128 - di, :], m_sA[di:128, :])
            nc.sync.dma_start(t[128 - di:128, :], m_sB[0:di, :])
            msd[di] = t

        # ---- phase A: per-tap weight planes + denominator (DVE + ACT) ----
        # Batched over the 5 dj taps of each di via overlapping-window APs.
        Wp = {}
        denom = pool_mean.tile([128, W], f32, name="denom")
        act_dis = sorted({t // 5 for t in act})
        first_di = act_dis[0]
        for di in act_dis:
            djs = [dj for dj in range(5) if (di * 5 + dj) in active_taps]
            dj0, ndj = djs[0], len(djs)
            # d5[p, j, w] = m_s_di[p, w+dj0+j] - m_s_c[p, w]: overlapping
            # windows along w expressed as a hand-built 3D AP (dj step 1).
            from concourse.ap import AP as _AP
            in0 = msd[di][:, dj0:dj0 + W + ndj - 1]
            in0w = _AP(in0.tensor, in0.offset,
                       [list(in0.ap[0]), [1, ndj], [1, W]])
            ctr = msd[2][:, 2:2 + W].unsqueeze(1).broadcast_to([128, ndj, W])
            d5 = pool_w.tile([128, ndj, W], f32, tag="d", name=f"d{di}")
            nc.vector.tensor_tensor(out=d5[:], in0=in0w, in1=ctr, op=AL.subtract)
            sq5 = pool_w.tile([128, ndj, W], f32, tag="sq", name=f"sq{di}")
            nc.scalar.activation(out=sq5[:], in_=d5[:], func=AF.Square)
            e5 = pool_w.tile([128, ndj, W], f32, tag="e", name=f"e{di}")
            nc.scalar.activation(out=e5[:], in_=sq5[:], func=AF.Exp, scale=-1.0)
            # W planes: one tile per di holding ndj planes; sk broadcast per tap
            W5 = pool_wp.tile([128, ndj, W], f32, tag=f"W{di}", bufs=1, name=f"W{di}")
            skb = cst[:, di * 5 + dj0: di * 5 + dj0 + ndj] \
                .unsqueeze(2).broadcast_to([128, ndj, W])
            nc.vector.tensor_tensor(out=W5[:], in0=e5[:], in1=skb, op=AL.mult)
            for j, dj in enumerate(djs):
                Wp[di * 5 + dj] = W5[:, j, :]
            # denom chain per tap (scalar per-partition STT)
            for j, dj in enumerate(djs):
                t_idx = di * 5 + dj
                if t_idx == t_first:
                    nc.vector.tensor_scalar_mul(
                        out=denom[:], in0=e5[:, j, :],
                        scalar1=cst[:, 25 + t_idx:26 + t_idx])
                else:
                    nc.vector.scalar_tensor_tensor(
                        out=denom[:], in0=e5[:, j, :],
                        scalar=cst[:, 25 + t_idx:26 + t_idx],
                        in1=denom[:], op0=AL.mult, op1=AL.add)
        recip = pool_mean.tile([128, W], f32, name="recip")
        nc.vector.reciprocal(out=recip[:], in_=denom[:])

        # ---- phase B: per-half MAC: DVE mults; adds split PE(psum)/DVE ----
        QP = 12           # channels accumulated on PE per half (6 banks)
        QD = HC - QP      # channels accumulated on DVE per half
        for half in range(2):
            ch0 = half * HC
            pb = [pool_ps.tile([128, BC, W], f32, tag=f"pb{b}", name=f"pb{half}_{b}")
                  for b in range(QP // BC)]
            acc_sb = pool_tmp.tile([128, QD, W], f32, tag="acc_sb", bufs=1, name=f"accs{half}")
            for di in range(5):
                if not any((di * 5 + dj) in active_taps for dj in range(5)):
                    continue
                xh = pool_x.tile([128, HC, SLAB_W], f32, tag="xh", name=f"xh{half}_{di}")
                nc.sync.dma_start(
                    xh[:], xs_d[ch0:ch0 + HC, di:di + 128, :].transpose([1, 0, 2]))
                for dj in range(5):
                    t_idx = di * 5 + dj
                    if t_idx not in active_taps:
                        continue
                    prod = pool_tmp.tile([128, HC, W], f32, tag="prod",
                                         name=f"pr{half}_{t_idx}")
                    nc.vector.tensor_tensor(
                        out=prod[:],
                        in0=Wp[t_idx].unsqueeze(1).broadcast_to([128, HC, W]),
                        in1=xh[:, :, dj:dj + W], op=AL.mult)
                    for b in range(QP // BC):
                        nc.tensor.matmul(
                            pb[b][:], ident[:], prod[:, b * BC:(b + 1) * BC, :],
                            start=(t_idx == t_first), stop=(t_idx == t_last))
                    if t_idx == t_first:
                        nc.vector.tensor_copy(acc_sb[:], prod[:, QP:HC, :])
                    else:
                        nc.vector.tensor_add(acc_sb[:], acc_sb[:], prod[:, QP:HC, :])
            og = pool_tmp.tile([128, HC, W], f32, tag="prod", name=f"og{half}")
            for b in range(QP // BC):
                nc.vector.tensor_tensor(
                    out=og[:, b * BC:(b + 1) * BC, :], in0=pb[b][:],
                    in1=recip[:].unsqueeze(1).broadcast_to([128, BC, W]),
                    op=AL.mult)
            nc.vector.tensor_tensor(
                out=og[:, QP:HC, :], in0=acc_sb[:],
                in1=recip[:].unsqueeze(1).broadcast_to([128, QD, W]), op=AL.mult)
            nc.sync.dma_start(
                out_d[ch0:ch0 + HC, :, :].transpose([1, 0, 2]), og[:])

    nc.compile()
    return nc


def _prep_inputs(x, params):
    """Build per-core in_maps."""
    x = np.ascontiguousarray(x, dtype=np.float32)
    tap_consts = _host_tap_constants(params)
    active = set()
    for (sk_eff, sk2, s2c, is5) in tap_consts:
        active |= {t for t in range(NT) if sk2[t] != 0.0}
    # pad whole batch once: [B, C, H+4, W+4]
    xp = np.pad(x, ((0, 0), (0, 0), (2, 2), (2, 2)))
    in_maps = []
    for core in range(NCORES):
        b, half = core // 2, core % 2
        h0 = half * HALF
        slab = np.ascontiguousarray(xp[b, :, h0:h0 + SLAB_H, :])
        sk_eff, sk2, s2c, _ = tap_consts[b]
        cst = np.zeros((128, 51), np.float32)
        cst[:, 0:25] = sk_eff[None, :]
        cst[:, 25:50] = sk2[None, :]
        cst[:, 50] = s2c
        in_maps.append({"xs": slab, "cst": cst,
                        "ident": np.eye(128, dtype=np.float32),
                        "ident4": np.repeat(np.eye(4, dtype=np.float32), 32, axis=0)})
    return in_maps, frozenset(active)


def kernel(x, params, n_iter=1, c0=C0):
    from concourse.bass_utils import run_bass_kernel_spmd
    in_maps, active = _prep_inputs(x, params)
    key = ("nc", active, n_iter, c0)
    if key not in _CACHE:
        _CACHE[key] = _build(active, n_iter, c0)
    nc = _CACHE[key]
    res = run_bass_kernel_spmd(nc, in_maps, list(range(NCORES)))
    out = np.empty((B, C, H, W), np.float32)
    for core in range(NCORES):
        b, half = core // 2, core % 2
        out[b, :, half * HALF:(half + 1) * HALF, :] = res.results[core]["out"]
    return out
